# revision 15
# baseline (speedup 1.0000x reference)
"""Trainium2 Bass kernel for nn_CustomLlamaAttention (partial-RoPE GQA attention
with low-rank KV, tensor-parallel over heads on 8 NeuronCores).

v5: host-side weight folding + collective/schedule restructure:
  - W_kc = upk_w @ down_w and W_v = upv_w @ down_w folded on host (exact in
    fp32): the latent path (down-proj shard, latent AllGather, up-proj
    matmuls) disappears. wcat = [wv 64 | kr 32 | wkc 32 | q 256] = exactly
    3 chunks of 128 columns.
  - V (s-major) via 16 PE transposes of the folded v^T rows (partition 0
    base); identity supplied from host.
  - tiny warm-up AllGather triggered at t~0 absorbs the first-collective
    init barrier during compute.
  - attention chunks processed in DESCENDING order (j=3..0) so the largest
    chunk's output AllGather overlaps the remaining attention and the last
    AG is the smallest amount of work from completion.
  - diagonal trimming: scores/exp/mask/AV are sliced to the causally valid
    q range per diagonal k-tile (PSUM accumulate composes across slices).
  - dependent DMAs (rope path, q swaps, og writes, tail) carry
    tile_wait_until hints so the scheduler never interleaves them into the
    bulk hs/wc input stream or hoists AG-dependent o-proj matmuls ahead of
    attention on the PE queue.

Sharding: core c owns q heads 4c..4c+3 (= KV head c), o_proj output rows
256c..256c+256; final [256, S] output shards concatenated on host.
"""

import sys

for _p in ("/opt/trn_rl_repo",):
    if _p not in sys.path:
        sys.path.append(_p)

import numpy as np
import ml_dtypes

import concourse.bass as bass
import concourse.tile as tile
from concourse import bacc
from concourse import mybir
from concourse.bass import ts
from concourse.bass_utils import run_bass_kernel_spmd

# ---- problem constants (hardcoded per spec) ----
HID = 2048
NH = 32
NKV = 8
HD = 64
LR = 32
TOPK = 16
THETA = 10000.0
B, S = 1, 2048
NCORES = 8
HPC = NH // NCORES          # 4 q heads per core
QSH = HPC * HD              # 256 q rows per core
KRSH = 2 * TOPK             # 32 roped dims per KV head
NOPESH = HD - KRSH          # 32 nope dims per KV head
LAT = LR * NKV              # 256 latent
WC = HD + KRSH + NOPESH + QSH  # 384 = [wv 64 | kr 32 | wkc 32 | q 256]
P = 128
NSC = S // 512              # 4 attention q chunks of 512
NKT = S // P                # 16 k-tiles of 128
BF = mybir.dt.bfloat16
F32 = mybir.dt.float32


def _build_program():
    nc = bacc.Bacc(
        "TRN2",
        target_bir_lowering=False,
        debug=False,
        num_devices=NCORES,
    )

    # DRAM I/O (per-core data supplied via in_maps)
    hsT_d = nc.dram_tensor("hsT", [HID, S], BF, kind="ExternalInput").ap()
    wcatT_d = nc.dram_tensor("wcatT", [HID, WC], BF, kind="ExternalInput").ap()
    owT_d = nc.dram_tensor("owT", [HID, QSH], BF, kind="ExternalInput").ap()
    cosq_d = nc.dram_tensor("cosq", [P, S], BF, kind="ExternalInput").ap()
    sinq_d = nc.dram_tensor("sinq", [P, S], BF, kind="ExternalInput").ap()
    cosk_d = nc.dram_tensor("cosk", [KRSH, S], BF, kind="ExternalInput").ap()
    sink_d = nc.dram_tensor("sink", [KRSH, S], BF, kind="ExternalInput").ap()
    mask_d = nc.dram_tensor("masktri", [P, P], BF, kind="ExternalInput").ap()
    ident_d = nc.dram_tensor("ident", [64, 64], BF, kind="ExternalInput").ap()

    outT_d = nc.dram_tensor("outT", [QSH, S], BF, kind="ExternalOutput").ap()

    # internal DRAM: per-512-chunk attention-output AllGathers + warm-up
    og_in = [nc.dram_tensor(f"og_in{j}", [QSH, 512], BF).ap() for j in range(NSC)]
    og_out = [
        nc.dram_tensor(f"og_out{j}", [NH * HD, 512], BF, addr_space="Shared").ap()
        for j in range(NSC)
    ]
    warm_in = nc.dram_tensor("warm_in", [1, 256], BF).ap()
    warm_out = nc.dram_tensor("warm_out", [NCORES, 256], BF, addr_space="Shared").ap()

    Exp = mybir.ActivationFunctionType.Exp
    JORDER = [3, 2, 1, 0]   # chunk processing order (big first)

    with tile.TileContext(nc) as tc:
        with (
            tc.tile_pool(name="sing", bufs=1) as sing,
            tc.tile_pool(name="tmp", bufs=2) as tmp,
            tc.tile_pool(name="qshp", bufs=2) as qshp,
            tc.tile_pool(name="apool", bufs=3) as apool,
            tc.tile_pool(name="otp", bufs=2) as otp,
            tc.tile_pool(name="ostp", bufs=2) as ostp,
            tc.tile_pool(name="nrm", bufs=1) as nrm,
            tc.tile_pool(name="psum_s", bufs=2, space="PSUM") as psum_s,
            tc.tile_pool(name="psum_mm", bufs=2, space="PSUM") as psum_mm,
            tc.tile_pool(name="psum_av", bufs=2, space="PSUM") as psum_av,
        ):
            # ---- persistent SBUF tiles ----
            hs = [
                sing.tile([P, S], BF, tag=f"hs{k}", name=f"hs{k}") for k in range(NKT)
            ]
            wc = [
                sing.tile([P, WC], BF, tag=f"wc{k}", name=f"wc{k}") for k in range(NKT)
            ]
            yT0 = sing.tile([P, S], BF, tag="yT0")  # rows 0:64 v | 64:96 kr | 96:128 kc
            krT = sing.tile([KRSH, S], BF, tag="krT")
            ident_sb = sing.tile([64, 64], BF, tag="ident")
            qT = sing.tile([P, 2, S], BF, tag="qT")
            qr0 = sing.tile([P, S], BF, tag="qr0")
            qr1 = sing.tile([P, S], BF, tag="qr1")
            krot = sing.tile([KRSH, S], BF, tag="krot")
            ksh = sing.tile([KRSH, S], BF, tag="ksh")
            cosq_sb = sing.tile([P, S], BF, tag="cosq")
            sinq_sb = sing.tile([P, S], BF, tag="sinq")
            cosk_sb = sing.tile([KRSH, S], BF, tag="cosk")
            sink_sb = sing.tile([KRSH, S], BF, tag="sink")
            mask_sb = sing.tile([P, P], BF, tag="mask")
            ow_sb = sing.tile([P, NKT, QSH], BF, tag="ow")
            KT = sing.tile([P, NKT, P], BF, tag="KT")    # 2 dup bands of 64 d
            V = sing.tile([P, NKT, HD + 1], BF, tag="V")  # col HD = ones

            # warm up the CC engine so the first real AllGather pays no
            # init-barrier cost; triggered immediately (no dependencies)
            nc.gpsimd.collective_compute(
                "AllGather",
                mybir.AluOpType.bypass,
                replica_groups=[list(range(NCORES))],
                ins=[warm_in],
                outs=[warm_out],
            )

            # ---- bulk input DMA stream on the sync queue (dep-free) ----
            nc.sync.dma_start(out=cosq_sb, in_=cosq_d)
            nc.sync.dma_start(out=sinq_sb, in_=sinq_d)
            nc.sync.dma_start(out=cosk_sb, in_=cosk_d)
            nc.sync.dma_start(out=sink_sb, in_=sink_d)
            for k in range(NKT):
                nc.sync.dma_start(out=wc[k], in_=wcatT_d[ts(k, P), :])
                nc.sync.dma_start(out=hs[k], in_=hsT_d[ts(k, P), :])
            # small extras on the scalar HWDGE queue
            nc.scalar.dma_start(out=mask_sb, in_=mask_d)
            nc.scalar.dma_start(out=ident_sb, in_=ident_d)
            nc.scalar.dma_start(
                out=ow_sb, in_=owT_d.rearrange("(ko p) m -> p ko m", p=P)
            )

            nc.vector.memset(V[:, :, HD : HD + 1], 1.0)

            # ---- phase 1: fused projection  y^T = wcat @ hs^T ----
            # 6 streamed chains per k-tile: dkv(scj 0..3) + q01(scj3) + q23(scj3)
            dkvA = psum_s.tile([P, 2, 512], F32, tag="s", name="ph1_dkvA")
            dkvB = psum_s.tile([P, 2, 512], F32, tag="s", name="ph1_dkvB")
            first_scj = JORDER[0]
            qps = {
                (0, first_scj): psum_mm.tile([P, 512], F32, tag="mm", name="q01_s3"),
                (1, first_scj): psum_mm.tile([P, 512], F32, tag="mm", name="q23_s3"),
            }

            def dkv_ps(scj):
                t = (dkvA, dkvB)[scj // 2]
                return t[:, scj % 2, :]

            for k in range(NKT):
                st, sp = (k == 0), (k == NKT - 1)
                for scj in range(NSC):
                    nc.tensor.matmul(
                        dkv_ps(scj),
                        lhsT=wc[k][:, 0:128],
                        rhs=hs[k][:, ts(scj, 512)],
                        start=st,
                        stop=sp,
                        skip_group_check=True,
                    )
                for p_i in (0, 1):
                    nc.tensor.matmul(
                        qps[(p_i, first_scj)],
                        lhsT=wc[k][:, 128 + p_i * 128 : 256 + p_i * 128],
                        rhs=hs[k][:, ts(first_scj, 512)],
                        start=st,
                        stop=sp,
                    )

            # copies out of psum: dkv -> yT0 (vector)
            for scj in range(NSC):
                nc.vector.tensor_copy(out=yT0[:, ts(scj, 512)], in_=dkv_ps(scj))

            # rope(kr): relocate kr rows to partition 0, then shifted-pair rope
            with tc.tile_wait_until(0.15):
                nc.sync.dma_start(out=krT, in_=yT0[64:96])
                nc.sync.dma_start(out=ksh[0:16], in_=krT[16:32])
                nc.sync.dma_start(out=ksh[16:32], in_=krT[0:16])
            nc.vector.tensor_mul(out=krot, in0=krT, in1=cosk_sb)
            nc.vector.tensor_mul(out=ksh, in0=ksh, in1=sink_sb)
            nc.vector.tensor_add(out=krot, in0=krot, in1=ksh)
            # scatter rope'd k rows into KT bands (d 0:16 and 32:48) and
            # nope rows (wkc) into bands (d 16:32, 48:64)
            with tc.tile_wait_until(0.16):
                for b in (0, 64):
                    nc.sync.dma_start(
                        out=KT[b + 0 : b + 16, :, :],
                        in_=krot[0:16].rearrange("p (ko ki) -> p ko ki", ki=P),
                    )
                    nc.sync.dma_start(
                        out=KT[b + 32 : b + 48, :, :],
                        in_=krot[16:32].rearrange("p (ko ki) -> p ko ki", ki=P),
                    )
                    nc.sync.dma_start(
                        out=KT[b + 16 : b + 32, :, :],
                        in_=yT0[96:112].rearrange("p (ko ki) -> p ko ki", ki=P),
                    )
                    nc.sync.dma_start(
                        out=KT[b + 48 : b + 64, :, :],
                        in_=yT0[112:128].rearrange("p (ko ki) -> p ko ki", ki=P),
                    )

            # V via PE transpose of v^T rows (yT0[0:64], partition base 0)
            for kt in range(NKT):
                tp = psum_av.tile([P, 1024], BF, tag="av", name=f"vtp_{kt}")
                nc.tensor.transpose(tp[:, 0:HD], yT0[0:64, ts(kt, P)], ident_sb)
                nc.vector.tensor_copy(out=V[:, kt, 0:HD], in_=tp[:, 0:HD])

            def q_copy_rope(scj, wait_ms):
                """qT copies (vector), swap DMAs, rope (vector) for one scj."""
                sl = ts(scj, 512)
                for p_i, qr in ((0, qr0), (1, qr1)):
                    nc.vector.tensor_copy(out=qT[:, p_i, sl], in_=qps[(p_i, scj)])
                    qsh = qshp.tile([P, 512], BF, tag="qsh")
                    qt = qT[:, p_i, :]
                    with tc.tile_wait_until(wait_ms):
                        for b in (0, 64):
                            nc.sync.dma_start(
                                out=qsh[b : b + 32, :], in_=qt[b + 32 : b + 64, sl]
                            )
                            nc.sync.dma_start(
                                out=qsh[b + 32 : b + 64, :], in_=qt[b : b + 32, sl]
                            )
                    nc.vector.tensor_mul(out=qr[:, sl], in0=qt[:, sl], in1=cosq_sb[:, sl])
                    nc.vector.tensor_mul(out=qsh, in0=qsh, in1=sinq_sb[:, sl])
                    nc.vector.tensor_add(out=qr[:, sl], in0=qr[:, sl], in1=qsh)

            q_copy_rope(first_scj, 0.17)

            # ---- phase 3: causal attention, chunks big-first ----
            # q-chain MMs for the NEXT chunk's scj are spread between slots.
            def emit_q_chain_mms(pending):
                if pending:
                    p_i, scj, k = pending.pop(0)
                    nc.tensor.matmul(
                        qps[(p_i, scj)],
                        lhsT=wc[k][:, 128 + p_i * 128 : 256 + p_i * 128],
                        rhs=hs[k][:, ts(scj, 512)],
                        start=(k == 0),
                        stop=(k == NKT - 1),
                    )

            for jidx, j in enumerate(JORDER):
                nkt = 4 * j + 4
                # prepare pending q-chain work for the next chunk in order
                pending = []
                if jidx + 1 < NSC:
                    scj = JORDER[jidx + 1]
                    qps[(0, scj)] = psum_mm.tile([P, 512], F32, tag="mm", name=f"q01_s{scj}")
                    qps[(1, scj)] = psum_mm.tile([P, 512], F32, tag="mm", name=f"q23_s{scj}")
                    for k in range(NKT):
                        pending.append((0, scj, k))
                        pending.append((1, scj, k))
                    per_slot = max(1, (len(pending) + 2 * nkt - 1) // (2 * nkt))
                else:
                    per_slot = 0

                for p_i, qr in enumerate((qr0, qr1)):
                    av = [
                        psum_av.tile([P, 512], F32, tag="av", name=f"av0_{j}_{p_i}"),
                        psum_av.tile([P, 512], F32, tag="av", name=f"av1_{j}_{p_i}"),
                    ]
                    for kt in range(nkt):
                        d = kt - 4 * j          # diagonal offset (>=0 on diag)
                        q0 = 128 * d if d > 0 else 0   # valid q start in chunk
                        w = 512 - q0
                        qsl = slice(512 * j + q0, 512 * j + 512)
                        ss = psum_s.tile(
                            [P, 2, 512], F32, tag="s", name=f"s_{j}_{p_i}_{kt}"
                        )
                        for hb, b0 in ((0, 0), (1, 64)):
                            nc.tensor.matmul(
                                ss[:, hb, q0:512],
                                lhsT=KT[b0 : b0 + 64, kt, :],
                                rhs=qr[b0 : b0 + 64, qsl],
                                start=True,
                                stop=True,
                                skip_group_check=True,
                            )
                        a = apool.tile([P, 2, 512], BF, tag="a")
                        nc.scalar.activation(a[:, :, q0:512], ss[:, :, q0:512], Exp)
                        if d >= 0:
                            # straddle block mask (triangular within 128 q)
                            for hb in (0, 1):
                                nc.vector.tensor_mul(
                                    out=a[:, hb, q0 : q0 + P],
                                    in0=a[:, hb, q0 : q0 + P],
                                    in1=mask_sb,
                                )
                        for hb in (0, 1):
                            nc.tensor.matmul(
                                av[hb][0 : HD + 1, q0:512],
                                lhsT=V[:, kt, :],
                                rhs=a[:, hb, q0:512],
                                start=(kt == 0),
                                stop=(kt == nkt - 1),
                            )
                        for _ in range(per_slot):
                            emit_q_chain_mms(pending)
                    for hb in (0, 1):
                        h = 2 * p_i + hb
                        # drain av psum to SBUF immediately so the slot frees
                        avc = tmp.tile([HD + 1, 512], F32, tag="avc")
                        nc.vector.tensor_copy(out=avc, in_=av[hb][0 : HD + 1, :])
                        dn = nrm.tile([1, 512], F32, tag="dn")
                        nc.vector.tensor_copy(out=dn, in_=avc[HD : HD + 1, :])
                        rc = nrm.tile([1, 512], F32, tag="rc")
                        nc.vector.reciprocal_approx_fast(rc, dn)
                        bc = nrm.tile([HD, 512], F32, tag="bc")
                        nc.gpsimd.partition_broadcast(bc, rc, channels=HD)
                        on = nrm.tile([HD, 512], BF, tag="on")
                        nc.vector.tensor_mul(out=on, in0=avc[0:HD, :], in1=bc)
                        with tc.tile_wait_until(0.2 + 0.02 * jidx):
                            nc.sync.dma_start(
                                out=og_in[j][h * HD : (h + 1) * HD, :], in_=on
                            )
                # flush remaining q-chain MMs, then copies/rope for next scj
                while pending:
                    emit_q_chain_mms(pending)
                if jidx + 1 < NSC:
                    q_copy_rope(JORDER[jidx + 1], 0.2 + 0.02 * jidx)
                # this chunk's attention-output AllGather
                nc.gpsimd.collective_compute(
                    "AllGather",
                    mybir.AluOpType.bypass,
                    replica_groups=[list(range(NCORES))],
                    ins=[og_in[j]],
                    outs=[og_out[j]],
                )

            # ---- tail: o-projection per gathered chunk (pinned after attn) ----
            for gi, j in enumerate(JORDER):
                wms = 1.0 + 0.1 * gi
                OT = otp.tile([P, NKT, 512], BF, tag="ot")
                with tc.tile_wait_until(wms):
                    nc.sync.dma_start(
                        out=OT,
                        in_=og_out[j].rearrange("(ko p) q -> p ko q", p=P),
                    )
                for mc in range(2):
                    ps = psum_mm.tile([P, 512], F32, tag="mm", name=f"op_{j}_{mc}")
                    with tc.tile_wait_until(wms):
                        for k in range(NKT):
                            nc.tensor.matmul(
                                ps,
                                lhsT=ow_sb[:, k, ts(mc, P)],
                                rhs=OT[:, k, :],
                                start=(k == 0),
                                stop=(k == NKT - 1),
                            )
                        ot = ostp.tile([P, 512], BF, tag="ost")
                        nc.vector.tensor_copy(out=ot, in_=ps)
                        nc.sync.dma_start(
                            out=outT_d[ts(mc, P), ts(j, 512)], in_=ot
                        )

    nc.compile()
    return nc


_NC_CACHE = None


def _get_program():
    global _NC_CACHE
    if _NC_CACHE is None:
        _NC_CACHE = _build_program()
    return _NC_CACHE


def _bf16(x):
    return np.asarray(x, dtype=np.float32).astype(ml_dtypes.bfloat16)


def _host_inputs(hidden_states, q_w, kr_w, down_w, upk_w, upv_w, o_w):
    hs = np.asarray(hidden_states, dtype=np.float32)[0]  # [S, HID]
    q_w = np.asarray(q_w, np.float32)
    kr_w = np.asarray(kr_w, np.float32)
    down_w = np.asarray(down_w, np.float32)
    upk_w = np.asarray(upk_w, np.float32)
    upv_w = np.asarray(upv_w, np.float32)
    o_w = np.asarray(o_w, np.float32)

    hsT = _bf16(hs.T)  # [HID, S]

    # fold the low-rank KV path on the host (exact in fp32)
    wkc = upk_w @ down_w   # [N_NOPE=256, HID]
    wv = upv_w @ down_w    # [NKV*HD=512, HID]

    # RoPE tables (fp32 host math, bf16 on device)
    pos = np.arange(S, dtype=np.float32)
    inv = 1.0 / (THETA ** (np.arange(0, HD, 2, dtype=np.float32) / HD))
    fr = pos[:, None] * inv[None, :]           # [S, 32]
    emb = np.concatenate([fr, fr], -1)         # [S, 64]
    cosT = np.cos(emb).T                       # [64, S]
    sinT = np.sin(emb).T
    sc = 1.0 / np.sqrt(np.float32(HD))

    cosq = np.tile(cosT, (2, 1)) * sc          # [128, S]
    sgn = np.where(np.arange(HD) < 32, -1.0, 1.0).astype(np.float32)[:, None]
    sinq = np.tile(sinT * sgn, (2, 1)) * sc    # [128, S]

    rope_d = np.concatenate([np.arange(0, 16), np.arange(32, 48)])
    cosk = cosT[rope_d]                        # [32, S]
    sgnk = np.where(np.arange(KRSH) < 16, -1.0, 1.0).astype(np.float32)[:, None]
    sink = sinT[rope_d] * sgnk

    # triangular straddle-block mask [128 k, 128 q]
    kk = np.arange(P)[:, None]
    qq = np.arange(P)[None, :]
    mask = (kk <= qq).astype(np.float32)

    shared = {
        "hsT": hsT,
        "cosq": _bf16(cosq),
        "sinq": _bf16(sinq),
        "cosk": _bf16(cosk),
        "sink": _bf16(sink),
        "masktri": _bf16(mask),
        "ident": _bf16(np.eye(64, dtype=np.float32)),
    }
    in_maps = []
    for c in range(NCORES):
        q_rows = q_w[c * QSH : (c + 1) * QSH]            # [256, HID]
        kr_rows = kr_w[c * KRSH : (c + 1) * KRSH]        # [32, HID]
        kc_rows = wkc[c * NOPESH : (c + 1) * NOPESH]     # [32, HID]
        v_rows = wv[c * HD : (c + 1) * HD]               # [64, HID]
        wcat = np.concatenate([v_rows, kr_rows, kc_rows, q_rows], axis=0)  # [384, HID]
        m = dict(shared)
        m["wcatT"] = _bf16(wcat.T)                       # [HID, 384]
        m["owT"] = _bf16(o_w[c * QSH : (c + 1) * QSH].T)  # [HID, 256]
        in_maps.append(m)
    return in_maps


def kernel(**inputs) -> np.ndarray:
    nc = _get_program()
    in_maps = _host_inputs(**inputs)
    res = run_bass_kernel_spmd(nc, in_maps, core_ids=list(range(NCORES)))
    outT = np.concatenate(
        [np.asarray(res.results[c]["outT"]) for c in range(NCORES)], axis=0
    )  # [2048, S] bf16
    return np.ascontiguousarray(outT.astype(np.float32).T)[None]


if __name__ == "__main__":
    rng = np.random.default_rng(0)
    ins = {
        "hidden_states": rng.standard_normal((B, S, HID), dtype=np.float32),
        "q_w": rng.standard_normal((NH * HD, HID), dtype=np.float32) * 0.02,
        "kr_w": rng.standard_normal((2 * TOPK * NKV, HID), dtype=np.float32) * 0.02,
        "down_w": rng.standard_normal((LAT, HID), dtype=np.float32) * 0.02,
        "upk_w": rng.standard_normal((NOPESH * NKV, LAT), dtype=np.float32) * 0.02,
        "upv_w": rng.standard_normal((NKV * HD, LAT), dtype=np.float32) * 0.02,
        "o_w": rng.standard_normal((HID, NH * HD), dtype=np.float32) * 0.02,
    }
    out = kernel(**ins)
    print(out.shape, out.dtype, float(np.abs(out).max()))


# revision 16
# speedup vs baseline: 1.0448x; 1.0448x over previous
"""Trainium2 Bass kernel for nn_CustomLlamaAttention (partial-RoPE GQA attention
with low-rank KV, tensor-parallel over heads on 8 NeuronCores).

v5: host-side weight folding + collective/schedule restructure:
  - W_kc = upk_w @ down_w and W_v = upv_w @ down_w folded on host (exact in
    fp32): the latent path (down-proj shard, latent AllGather, up-proj
    matmuls) disappears. wcat = [wv 64 | kr 32 | wkc 32 | q 256] = exactly
    3 chunks of 128 columns.
  - V (s-major) via 16 PE transposes of the folded v^T rows (partition 0
    base); identity supplied from host.
  - tiny warm-up AllGather triggered at t~0 absorbs the first-collective
    init barrier during compute.
  - attention chunks processed in DESCENDING order (j=3..0) so the largest
    chunk's output AllGather overlaps the remaining attention and the last
    AG is the smallest amount of work from completion.
  - diagonal trimming: scores/exp/mask/AV are sliced to the causally valid
    q range per diagonal k-tile (PSUM accumulate composes across slices).
  - dependent DMAs (rope path, q swaps, og writes, tail) carry
    tile_wait_until hints so the scheduler never interleaves them into the
    bulk hs/wc input stream or hoists AG-dependent o-proj matmuls ahead of
    attention on the PE queue.

Sharding: core c owns q heads 4c..4c+3 (= KV head c), o_proj output rows
256c..256c+256; final [256, S] output shards concatenated on host.
"""

import sys

for _p in ("/opt/trn_rl_repo",):
    if _p not in sys.path:
        sys.path.append(_p)

import numpy as np
import ml_dtypes

import concourse.bass as bass
import concourse.tile as tile
from concourse import bacc
from concourse import mybir
from concourse.bass import ts
from concourse.bass_utils import run_bass_kernel_spmd

# ---- problem constants (hardcoded per spec) ----
HID = 2048
NH = 32
NKV = 8
HD = 64
LR = 32
TOPK = 16
THETA = 10000.0
B, S = 1, 2048
NCORES = 8
HPC = NH // NCORES          # 4 q heads per core
QSH = HPC * HD              # 256 q rows per core
KRSH = 2 * TOPK             # 32 roped dims per KV head
NOPESH = HD - KRSH          # 32 nope dims per KV head
LAT = LR * NKV              # 256 latent
WC = HD + KRSH + NOPESH + QSH  # 384 = [wv 64 | kr 32 | wkc 32 | q 256]
P = 128
NSC = S // 512              # 4 attention q chunks of 512
NKT = S // P                # 16 k-tiles of 128
BF = mybir.dt.bfloat16
F32 = mybir.dt.float32


def _build_program():
    nc = bacc.Bacc(
        "TRN2",
        target_bir_lowering=False,
        debug=False,
        num_devices=NCORES,
    )

    # DRAM I/O (per-core data supplied via in_maps)
    hsT_d = nc.dram_tensor("hsT", [HID, S], BF, kind="ExternalInput").ap()
    wcatT_d = nc.dram_tensor("wcatT", [HID, WC], BF, kind="ExternalInput").ap()
    owT_d = nc.dram_tensor("owT", [HID, QSH], BF, kind="ExternalInput").ap()
    cosq_d = nc.dram_tensor("cosq", [P, S], BF, kind="ExternalInput").ap()
    sinq_d = nc.dram_tensor("sinq", [P, S], BF, kind="ExternalInput").ap()
    cosk_d = nc.dram_tensor("cosk", [KRSH, S], BF, kind="ExternalInput").ap()
    sink_d = nc.dram_tensor("sink", [KRSH, S], BF, kind="ExternalInput").ap()
    mask_d = nc.dram_tensor("masktri", [P, P], BF, kind="ExternalInput").ap()
    ident_d = nc.dram_tensor("ident", [64, 64], BF, kind="ExternalInput").ap()

    outT_d = nc.dram_tensor("outT", [QSH, S], BF, kind="ExternalOutput").ap()

    # internal DRAM: per-512-chunk attention-output AllGathers + warm-up
    og_in = [nc.dram_tensor(f"og_in{j}", [QSH, 512], BF).ap() for j in range(NSC)]
    og_out = [
        nc.dram_tensor(f"og_out{j}", [NH * HD, 512], BF, addr_space="Shared").ap()
        for j in range(NSC)
    ]
    warm_in = nc.dram_tensor("warm_in", [1, 256], BF).ap()
    warm_out = nc.dram_tensor("warm_out", [NCORES, 256], BF, addr_space="Shared").ap()

    Exp = mybir.ActivationFunctionType.Exp
    JORDER = [3, 2, 1, 0]   # chunk processing order (big first)

    with tile.TileContext(nc) as tc:
        with (
            tc.tile_pool(name="sing", bufs=1) as sing,
            tc.tile_pool(name="tmp", bufs=2) as tmp,
            tc.tile_pool(name="qshp", bufs=2) as qshp,
            tc.tile_pool(name="apool", bufs=3) as apool,
            tc.tile_pool(name="otp", bufs=2) as otp,
            tc.tile_pool(name="ostp", bufs=2) as ostp,
            tc.tile_pool(name="nrm", bufs=1) as nrm,
            tc.tile_pool(name="psum_s", bufs=2, space="PSUM") as psum_s,
            tc.tile_pool(name="psum_mm", bufs=2, space="PSUM") as psum_mm,
            tc.tile_pool(name="psum_av", bufs=2, space="PSUM") as psum_av,
        ):
            # ---- persistent SBUF tiles ----
            hs = [
                sing.tile([P, S], BF, tag=f"hs{k}", name=f"hs{k}") for k in range(NKT)
            ]
            wc = [
                sing.tile([P, WC], BF, tag=f"wc{k}", name=f"wc{k}") for k in range(NKT)
            ]
            yT0 = sing.tile([P, S], BF, tag="yT0")  # rows 0:64 v | 64:96 kr | 96:128 kc
            krT = sing.tile([KRSH, S], BF, tag="krT")
            ident_sb = sing.tile([64, 64], BF, tag="ident")
            qT = sing.tile([P, 2, S], BF, tag="qT")
            qr0 = sing.tile([P, S], BF, tag="qr0")
            qr1 = sing.tile([P, S], BF, tag="qr1")
            krot = sing.tile([KRSH, S], BF, tag="krot")
            ksh = sing.tile([KRSH, S], BF, tag="ksh")
            cosq_sb = sing.tile([P, S], BF, tag="cosq")
            sinq_sb = sing.tile([P, S], BF, tag="sinq")
            cosk_sb = sing.tile([KRSH, S], BF, tag="cosk")
            sink_sb = sing.tile([KRSH, S], BF, tag="sink")
            mask_sb = sing.tile([P, P], BF, tag="mask")
            ow_sb = sing.tile([P, NKT, QSH], BF, tag="ow")
            KT = sing.tile([P, NKT, P], BF, tag="KT")    # 2 dup bands of 64 d
            V = sing.tile([P, NKT, HD + 1], BF, tag="V")  # col HD = ones

            # warm up the CC engine so the first real AllGather pays no
            # init-barrier cost; triggered immediately (no dependencies)
            nc.gpsimd.collective_compute(
                "AllGather",
                mybir.AluOpType.bypass,
                replica_groups=[list(range(NCORES))],
                ins=[warm_in],
                outs=[warm_out],
            )

            # ---- bulk input DMA stream on the sync queue (dep-free) ----
            nc.sync.dma_start(out=cosq_sb, in_=cosq_d)
            nc.sync.dma_start(out=sinq_sb, in_=sinq_d)
            nc.sync.dma_start(out=cosk_sb, in_=cosk_d)
            nc.sync.dma_start(out=sink_sb, in_=sink_d)
            for k in range(NKT):
                nc.sync.dma_start(out=wc[k], in_=wcatT_d[ts(k, P), :])
                nc.sync.dma_start(out=hs[k], in_=hsT_d[ts(k, P), :])
            # small extras on the scalar HWDGE queue
            nc.scalar.dma_start(out=mask_sb, in_=mask_d)
            nc.scalar.dma_start(out=ident_sb, in_=ident_d)
            nc.scalar.dma_start(
                out=ow_sb, in_=owT_d.rearrange("(ko p) m -> p ko m", p=P)
            )

            nc.vector.memset(V[:, :, HD : HD + 1], 1.0)

            # ---- phase 1: fused projection  y^T = wcat @ hs^T ----
            # 6 streamed chains per k-tile: dkv(scj 0..3) + q01(scj3) + q23(scj3)
            dkvA = psum_s.tile([P, 2, 512], F32, tag="s", name="ph1_dkvA")
            dkvB = psum_s.tile([P, 2, 512], F32, tag="s", name="ph1_dkvB")
            first_scj = JORDER[0]
            qps = {
                (0, first_scj): psum_mm.tile([P, 512], F32, tag="mm", name="q01_s3"),
                (1, first_scj): psum_mm.tile([P, 512], F32, tag="mm", name="q23_s3"),
            }

            def dkv_ps(scj):
                t = (dkvA, dkvB)[scj // 2]
                return t[:, scj % 2, :]

            for k in range(NKT):
                st, sp = (k == 0), (k == NKT - 1)
                for scj in range(NSC):
                    nc.tensor.matmul(
                        dkv_ps(scj),
                        lhsT=wc[k][:, 0:128],
                        rhs=hs[k][:, ts(scj, 512)],
                        start=st,
                        stop=sp,
                        skip_group_check=True,
                    )
                for p_i in (0, 1):
                    nc.tensor.matmul(
                        qps[(p_i, first_scj)],
                        lhsT=wc[k][:, 128 + p_i * 128 : 256 + p_i * 128],
                        rhs=hs[k][:, ts(first_scj, 512)],
                        start=st,
                        stop=sp,
                    )

            # copies out of psum: dkv -> yT0 (vector)
            for scj in range(NSC):
                nc.vector.tensor_copy(out=yT0[:, ts(scj, 512)], in_=dkv_ps(scj))

            # rope(kr): relocate kr rows to partition 0, then shifted-pair rope
            with tc.tile_wait_until(2.0):
                nc.sync.dma_start(out=krT, in_=yT0[64:96])
                nc.sync.dma_start(out=ksh[0:16], in_=krT[16:32])
                nc.sync.dma_start(out=ksh[16:32], in_=krT[0:16])
            nc.vector.tensor_mul(out=krot, in0=krT, in1=cosk_sb)
            nc.vector.tensor_mul(out=ksh, in0=ksh, in1=sink_sb)
            nc.vector.tensor_add(out=krot, in0=krot, in1=ksh)
            # scatter rope'd k rows into KT bands (d 0:16 and 32:48) and
            # nope rows (wkc) into bands (d 16:32, 48:64)
            with tc.tile_wait_until(2.05):
                for b in (0, 64):
                    nc.sync.dma_start(
                        out=KT[b + 0 : b + 16, :, :],
                        in_=krot[0:16].rearrange("p (ko ki) -> p ko ki", ki=P),
                    )
                    nc.sync.dma_start(
                        out=KT[b + 32 : b + 48, :, :],
                        in_=krot[16:32].rearrange("p (ko ki) -> p ko ki", ki=P),
                    )
                    nc.sync.dma_start(
                        out=KT[b + 16 : b + 32, :, :],
                        in_=yT0[96:112].rearrange("p (ko ki) -> p ko ki", ki=P),
                    )
                    nc.sync.dma_start(
                        out=KT[b + 48 : b + 64, :, :],
                        in_=yT0[112:128].rearrange("p (ko ki) -> p ko ki", ki=P),
                    )

            # V via PE transpose of v^T rows (yT0[0:64], partition base 0)
            for kt in range(NKT):
                tp = psum_av.tile([P, 1024], BF, tag="av", name=f"vtp_{kt}")
                nc.tensor.transpose(tp[:, 0:HD], yT0[0:64, ts(kt, P)], ident_sb)
                nc.vector.tensor_copy(out=V[:, kt, 0:HD], in_=tp[:, 0:HD])

            def q_copy_rope(scj, wait_ms):
                """qT copies (vector), swap DMAs, rope (vector) for one scj."""
                sl = ts(scj, 512)
                for p_i, qr in ((0, qr0), (1, qr1)):
                    nc.vector.tensor_copy(out=qT[:, p_i, sl], in_=qps[(p_i, scj)])
                    qsh = qshp.tile([P, 512], BF, tag="qsh")
                    qt = qT[:, p_i, :]
                    with tc.tile_wait_until(wait_ms):
                        for b in (0, 64):
                            nc.sync.dma_start(
                                out=qsh[b : b + 32, :], in_=qt[b + 32 : b + 64, sl]
                            )
                            nc.sync.dma_start(
                                out=qsh[b + 32 : b + 64, :], in_=qt[b : b + 32, sl]
                            )
                    nc.vector.tensor_mul(out=qr[:, sl], in0=qt[:, sl], in1=cosq_sb[:, sl])
                    nc.vector.tensor_mul(out=qsh, in0=qsh, in1=sinq_sb[:, sl])
                    nc.vector.tensor_add(out=qr[:, sl], in0=qr[:, sl], in1=qsh)

            q_copy_rope(first_scj, 2.1)

            # ---- phase 3: causal attention, chunks big-first ----
            # q-chain MMs for the NEXT chunk's scj are spread between slots.
            def emit_q_chain_mms(pending):
                if pending:
                    p_i, scj, k = pending.pop(0)
                    nc.tensor.matmul(
                        qps[(p_i, scj)],
                        lhsT=wc[k][:, 128 + p_i * 128 : 256 + p_i * 128],
                        rhs=hs[k][:, ts(scj, 512)],
                        start=(k == 0),
                        stop=(k == NKT - 1),
                    )

            for jidx, j in enumerate(JORDER):
                nkt = 4 * j + 4
                # prepare pending q-chain work for the next chunk in order
                pending = []
                if jidx + 1 < NSC:
                    scj = JORDER[jidx + 1]
                    qps[(0, scj)] = psum_mm.tile([P, 512], F32, tag="mm", name=f"q01_s{scj}")
                    qps[(1, scj)] = psum_mm.tile([P, 512], F32, tag="mm", name=f"q23_s{scj}")
                    for k in range(NKT):
                        pending.append((0, scj, k))
                        pending.append((1, scj, k))
                    per_slot = max(1, (len(pending) + 2 * nkt - 1) // (2 * nkt))
                else:
                    per_slot = 0

                for p_i, qr in enumerate((qr0, qr1)):
                    av = [
                        psum_av.tile([P, 512], F32, tag="av", name=f"av0_{j}_{p_i}"),
                        psum_av.tile([P, 512], F32, tag="av", name=f"av1_{j}_{p_i}"),
                    ]
                    for kt in range(nkt):
                        d = kt - 4 * j          # diagonal offset (>=0 on diag)
                        q0 = 128 * d if d > 0 else 0   # valid q start in chunk
                        w = 512 - q0
                        qsl = slice(512 * j + q0, 512 * j + 512)
                        ss = psum_s.tile(
                            [P, 2, 512], F32, tag="s", name=f"s_{j}_{p_i}_{kt}"
                        )
                        for hb, b0 in ((0, 0), (1, 64)):
                            nc.tensor.matmul(
                                ss[:, hb, q0:512],
                                lhsT=KT[b0 : b0 + 64, kt, :],
                                rhs=qr[b0 : b0 + 64, qsl],
                                start=True,
                                stop=True,
                                skip_group_check=True,
                            )
                        a = apool.tile([P, 2, 512], BF, tag="a")
                        nc.scalar.activation(a[:, :, q0:512], ss[:, :, q0:512], Exp)
                        if d >= 0:
                            # straddle block mask (triangular within 128 q)
                            for hb in (0, 1):
                                nc.vector.tensor_mul(
                                    out=a[:, hb, q0 : q0 + P],
                                    in0=a[:, hb, q0 : q0 + P],
                                    in1=mask_sb,
                                )
                        for hb in (0, 1):
                            nc.tensor.matmul(
                                av[hb][0 : HD + 1, q0:512],
                                lhsT=V[:, kt, :],
                                rhs=a[:, hb, q0:512],
                                start=(kt == 0),
                                stop=(kt == nkt - 1),
                            )
                        for _ in range(per_slot):
                            emit_q_chain_mms(pending)
                    for hb in (0, 1):
                        h = 2 * p_i + hb
                        # drain av psum to SBUF immediately so the slot frees
                        avc = tmp.tile([HD + 1, 512], F32, tag="avc")
                        nc.vector.tensor_copy(out=avc, in_=av[hb][0 : HD + 1, :])
                        dn = nrm.tile([1, 512], F32, tag="dn")
                        nc.vector.tensor_copy(out=dn, in_=avc[HD : HD + 1, :])
                        rc = nrm.tile([1, 512], F32, tag="rc")
                        nc.vector.reciprocal_approx_fast(rc, dn)
                        bc = nrm.tile([HD, 512], F32, tag="bc")
                        nc.gpsimd.partition_broadcast(bc, rc, channels=HD)
                        on = nrm.tile([HD, 512], BF, tag="on")
                        nc.vector.tensor_mul(out=on, in0=avc[0:HD, :], in1=bc)
                        with tc.tile_wait_until(3.1 + 0.3 * jidx):
                            nc.sync.dma_start(
                                out=og_in[j][h * HD : (h + 1) * HD, :], in_=on
                            )
                # flush remaining q-chain MMs, then copies/rope for next scj
                while pending:
                    emit_q_chain_mms(pending)
                if jidx + 1 < NSC:
                    q_copy_rope(JORDER[jidx + 1], 3.0 + 0.3 * jidx)
                # this chunk's attention-output AllGather
                nc.gpsimd.collective_compute(
                    "AllGather",
                    mybir.AluOpType.bypass,
                    replica_groups=[list(range(NCORES))],
                    ins=[og_in[j]],
                    outs=[og_out[j]],
                )

            # ---- tail: o-projection per gathered chunk (pinned after attn) ----
            for gi, j in enumerate(JORDER):
                wms = 6.0 + 0.2 * gi
                OT = otp.tile([P, NKT, 512], BF, tag="ot")
                with tc.tile_wait_until(wms):
                    nc.sync.dma_start(
                        out=OT,
                        in_=og_out[j].rearrange("(ko p) q -> p ko q", p=P),
                    )
                for mc in range(2):
                    ps = psum_mm.tile([P, 512], F32, tag="mm", name=f"op_{j}_{mc}")
                    with tc.tile_wait_until(wms):
                        for k in range(NKT):
                            nc.tensor.matmul(
                                ps,
                                lhsT=ow_sb[:, k, ts(mc, P)],
                                rhs=OT[:, k, :],
                                start=(k == 0),
                                stop=(k == NKT - 1),
                            )
                        ot = ostp.tile([P, 512], BF, tag="ost")
                        nc.vector.tensor_copy(out=ot, in_=ps)
                        nc.sync.dma_start(
                            out=outT_d[ts(mc, P), ts(j, 512)], in_=ot
                        )

    nc.compile()
    return nc


_NC_CACHE = None


def _get_program():
    global _NC_CACHE
    if _NC_CACHE is None:
        _NC_CACHE = _build_program()
    return _NC_CACHE


def _bf16(x):
    return np.asarray(x, dtype=np.float32).astype(ml_dtypes.bfloat16)


def _host_inputs(hidden_states, q_w, kr_w, down_w, upk_w, upv_w, o_w):
    hs = np.asarray(hidden_states, dtype=np.float32)[0]  # [S, HID]
    q_w = np.asarray(q_w, np.float32)
    kr_w = np.asarray(kr_w, np.float32)
    down_w = np.asarray(down_w, np.float32)
    upk_w = np.asarray(upk_w, np.float32)
    upv_w = np.asarray(upv_w, np.float32)
    o_w = np.asarray(o_w, np.float32)

    hsT = _bf16(hs.T)  # [HID, S]

    # fold the low-rank KV path on the host (exact in fp32)
    wkc = upk_w @ down_w   # [N_NOPE=256, HID]
    wv = upv_w @ down_w    # [NKV*HD=512, HID]

    # RoPE tables (fp32 host math, bf16 on device)
    pos = np.arange(S, dtype=np.float32)
    inv = 1.0 / (THETA ** (np.arange(0, HD, 2, dtype=np.float32) / HD))
    fr = pos[:, None] * inv[None, :]           # [S, 32]
    emb = np.concatenate([fr, fr], -1)         # [S, 64]
    cosT = np.cos(emb).T                       # [64, S]
    sinT = np.sin(emb).T
    sc = 1.0 / np.sqrt(np.float32(HD))

    cosq = np.tile(cosT, (2, 1)) * sc          # [128, S]
    sgn = np.where(np.arange(HD) < 32, -1.0, 1.0).astype(np.float32)[:, None]
    sinq = np.tile(sinT * sgn, (2, 1)) * sc    # [128, S]

    rope_d = np.concatenate([np.arange(0, 16), np.arange(32, 48)])
    cosk = cosT[rope_d]                        # [32, S]
    sgnk = np.where(np.arange(KRSH) < 16, -1.0, 1.0).astype(np.float32)[:, None]
    sink = sinT[rope_d] * sgnk

    # triangular straddle-block mask [128 k, 128 q]
    kk = np.arange(P)[:, None]
    qq = np.arange(P)[None, :]
    mask = (kk <= qq).astype(np.float32)

    shared = {
        "hsT": hsT,
        "cosq": _bf16(cosq),
        "sinq": _bf16(sinq),
        "cosk": _bf16(cosk),
        "sink": _bf16(sink),
        "masktri": _bf16(mask),
        "ident": _bf16(np.eye(64, dtype=np.float32)),
    }
    in_maps = []
    for c in range(NCORES):
        q_rows = q_w[c * QSH : (c + 1) * QSH]            # [256, HID]
        kr_rows = kr_w[c * KRSH : (c + 1) * KRSH]        # [32, HID]
        kc_rows = wkc[c * NOPESH : (c + 1) * NOPESH]     # [32, HID]
        v_rows = wv[c * HD : (c + 1) * HD]               # [64, HID]
        wcat = np.concatenate([v_rows, kr_rows, kc_rows, q_rows], axis=0)  # [384, HID]
        m = dict(shared)
        m["wcatT"] = _bf16(wcat.T)                       # [HID, 384]
        m["owT"] = _bf16(o_w[c * QSH : (c + 1) * QSH].T)  # [HID, 256]
        in_maps.append(m)
    return in_maps


def kernel(**inputs) -> np.ndarray:
    nc = _get_program()
    in_maps = _host_inputs(**inputs)
    res = run_bass_kernel_spmd(nc, in_maps, core_ids=list(range(NCORES)))
    outT = np.concatenate(
        [np.asarray(res.results[c]["outT"]) for c in range(NCORES)], axis=0
    )  # [2048, S] bf16
    return np.ascontiguousarray(outT.astype(np.float32).T)[None]


if __name__ == "__main__":
    rng = np.random.default_rng(0)
    ins = {
        "hidden_states": rng.standard_normal((B, S, HID), dtype=np.float32),
        "q_w": rng.standard_normal((NH * HD, HID), dtype=np.float32) * 0.02,
        "kr_w": rng.standard_normal((2 * TOPK * NKV, HID), dtype=np.float32) * 0.02,
        "down_w": rng.standard_normal((LAT, HID), dtype=np.float32) * 0.02,
        "upk_w": rng.standard_normal((NOPESH * NKV, LAT), dtype=np.float32) * 0.02,
        "upv_w": rng.standard_normal((NKV * HD, LAT), dtype=np.float32) * 0.02,
        "o_w": rng.standard_normal((HID, NH * HD), dtype=np.float32) * 0.02,
    }
    out = kernel(**ins)
    print(out.shape, out.dtype, float(np.abs(out).max()))
